# revision 77
# baseline (speedup 1.0000x reference)
"""Multi-head attention (B=4, S=1024, H=1024, heads=16) on 8 trn2 NeuronCores.

Sharding: data-parallel over batch (4) x tensor-parallel over head-groups (2).
Core c handles batch c//2, heads [8*(c%2), 8*(c%2)+8).

Schedule (all engines are in-order, so emission order is the schedule):
  - few, large DMAs (whole-tensor loads; ~0.6us sync-ring slot per DMA)
    ordered by need: vT/wv -> wq[pr0]/qT -> wk[pr0]/kT -> exp(bias) stream;
    later pairs' weight chunks load lazily from inside the attention loop.
  - a train of tiny warm-up matmuls during the initial DMA wait holds the
    PE HAM clock-gate at 8/8 so the real stream starts at 2.4 GHz.
  - per-pair attention slots: scores (row-group alternated so LDWEIGHTS
    hides), one exp + one multiply per [128,1024] tile, ctx matmuls lagged
    behind the exp/mul chain; projection bursts for the NEXT pair fill the
    tensor queue inside the slots.
  - pair boundaries: the previous pair's trailing ctx + normalize copies
    are emitted after the next pair's first scores; reciprocal+muls are
    deferred further so DMA-broadcast latency never blocks the vector queue.
  - output projection accumulates pr0-2 first (hiding pr3's normalize),
    and pr3 contributes via two base-0 K=64 halves (split wo tiles) so no
    partition-shift DMA sits on the critical tail.

Per-core math (all matmuls bf16 with fp32 PSUM accumulation):
  - projections: qh_T/kh_T in [d, i] layout (head dim on partitions), vh in
    [j, hd] layout augmented with a ones column per head (softmax
    denominator comes for free from the ctx matmul).
  - scores computed transposed (keys on partitions): s_T = khT^T @ qhT,
    exp on ScalarE, multiplied by host-precomputed exp(attn_bias)^T.
  - normalize with reciprocal of the denominator row + DMA partition
    broadcast, output projection with row-parallel Wo; host adds the two
    partial results + bo.

Scale (1/8) is folded into Wq/bq on the host. Softmax max-subtraction is
skipped: scores+bias are within +-8 so exp is well-conditioned in fp32.
"""

import numpy as np
import ml_dtypes

BF16 = ml_dtypes.bfloat16

S = 1024
HID = 1024
GCOL = 512  # hidden cols per core (8 heads * 64)
DH = 64
P = 128
NPAIR = 4  # head pairs per core
NJB = 8  # key blocks of 128
NCB = 8  # contraction blocks of 128
NIB = 8  # query blocks of 128

_CACHED_NC = None


def _build_nc():
    import concourse.bass as bass
    import concourse.mybir as mybir
    import concourse.tile as tile
    from concourse import bacc
    from contextlib import ExitStack

    f32 = mybir.dt.float32
    bf16 = mybir.dt.bfloat16
    AF = mybir.ActivationFunctionType

    nc = bacc.Bacc(
        "TRN2",
        target_bir_lowering=False,
        debug=False,
        enable_asserts=False,
        num_devices=8,
    )

    qT = nc.dram_tensor("qT", [HID, S], bf16, kind="ExternalInput").ap()
    kT = nc.dram_tensor("kT", [HID, S], bf16, kind="ExternalInput").ap()
    vT = nc.dram_tensor("vT", [HID, S], bf16, kind="ExternalInput").ap()
    # wq/wk host-chunked as [pr, p, cb, m] so per-head-pair weight loads are
    # contiguous 2KB-per-partition transfers (pr0's slice can load first)
    wq = nc.dram_tensor("wq", [NPAIR, P, NCB, P], bf16, kind="ExternalInput").ap()
    wk = nc.dram_tensor("wk", [NPAIR, P, NCB, P], bf16, kind="ExternalInput").ap()
    wv = nc.dram_tensor("wv", [HID, GCOL], bf16, kind="ExternalInput").ap()
    wo = nc.dram_tensor("wo", [GCOL, HID], bf16, kind="ExternalInput").ap()
    bq = nc.dram_tensor("bq", [GCOL], f32, kind="ExternalInput").ap()
    bk = nc.dram_tensor("bk", [GCOL], f32, kind="ExternalInput").ap()
    bv = nc.dram_tensor("bv", [GCOL], bf16, kind="ExternalInput").ap()
    # exp(bias) host-chunked per (pair, key block): one 512KB DMA each
    expb = nc.dram_tensor(
        "expb", [NPAIR, NJB, 2, P, S], bf16, kind="ExternalInput"
    ).ap()
    out = nc.dram_tensor("out", [S, HID], f32, kind="ExternalOutput").ap()

    with tile.TileContext(nc) as tc, ExitStack() as ctx:
        const = ctx.enter_context(tc.tile_pool(name="const", bufs=1))
        inT = ctx.enter_context(tc.tile_pool(name="inT", bufs=1))
        proj = ctx.enter_context(tc.tile_pool(name="proj", bufs=1))
        work = ctx.enter_context(tc.tile_pool(name="work", bufs=6))
        outp = ctx.enter_context(tc.tile_pool(name="outp", bufs=2))
        psum = ctx.enter_context(tc.tile_pool(name="psum", bufs=2, space="PSUM"))

        # ---- constants / weights ----
        wq_sb = const.tile([P, NPAIR, NCB, P], bf16, tag="wq")
        wk_sb = const.tile([P, NPAIR, NCB, P], bf16, tag="wk")
        wv_sb = const.tile([P, NCB, GCOL], bf16, tag="wv")
        wo_sb = const.tile([P, 3, HID], bf16, tag="wo")
        wv_r = wv.rearrange("(cb p) n -> p cb n", p=P)
        bq_sb = const.tile([P, NPAIR], f32, tag="bq")
        bk_sb = const.tile([P, NPAIR], f32, tag="bk")
        bv_sb = const.tile([1, GCOL], bf16, tag="bv")
        ones_k1 = const.tile([1, P], bf16, tag="ones_k1")
        nc.vector.memset(ones_k1, 1.0)
        # pre-warm the Exp activation table before the attention phase
        warm = const.tile([1, 16], bf16, tag="warm")
        nc.vector.memset(warm, 0.0)
        nc.scalar.activation(warm, warm, AF.Exp)

        qhT = [proj.tile([P, S], bf16, name=f"qhT{i}", tag=f"qhT{i}") for i in range(NPAIR)]
        khT = [proj.tile([P, S], bf16, name=f"khT{i}", tag=f"khT{i}") for i in range(NPAIR)]
        # vh_sb[jb]: [j in block, head, 65] where col 64 is ones (denominator trick)
        vh_sb = [proj.tile([P, 8, DH + 1], bf16, name=f"vh{i}", tag=f"vh{i}") for i in range(NJB)]
        ctxn = [proj.tile([P, S], bf16, name=f"ctxn{i}", tag=f"ctxn{i}") for i in range(NPAIR)]

        # input DMA order: v first (v_proj then overlaps the q/k stream),
        # then pr0's q/k weight chunks + the full q/k inputs. Whole-tensor
        # transfers: the sync DMA ring costs ~0.6us per instruction, so few
        # big DMAs beat many small ones.
        nc.sync.dma_start(out=bv_sb, in_=bv.rearrange("(a n) -> a n", a=1))
        nc.sync.dma_start(out=bq_sb, in_=bq.rearrange("(pr p) -> p pr", p=P))
        nc.sync.dma_start(out=bk_sb, in_=bk.rearrange("(pr p) -> p pr", p=P))
        # q first: its projection bursts run during the DMA wait (replacing
        # warm-up junk), then v, then k (gated by its later arrival anyway)
        qk_tiles = {}
        qin = inT.tile([P, NCB, S], bf16, name="qin", tag="qin")
        qin_r = qT.rearrange("(cb p) n -> p cb n", p=P)
        nc.sync.dma_start(out=wq_sb[:, 0], in_=wq[0])
        nc.sync.dma_start(out=qin[:, 0:4, :], in_=qin_r[:, 0:4, :])
        nc.sync.dma_start(out=qin[:, 4:8, :], in_=qin_r[:, 4:8, :])
        qk_tiles["q"] = [qin[:, cb, :] for cb in range(NCB)]
        nc.sync.dma_start(out=wv_sb, in_=wv.rearrange("(cb p) n -> p cb n", p=P))
        vin = inT.tile([P, NCB, S], bf16, name="vin", tag="vin")
        vin_r = vT.rearrange("(cb p) n -> p cb n", p=P)
        nc.sync.dma_start(out=vin[:, 0:4, :], in_=vin_r[:, 0:4, :])
        nc.sync.dma_start(out=vin[:, 4:8, :], in_=vin_r[:, 4:8, :])
        vtiles = [vin[:, cb, :] for cb in range(NCB)]
        kin = inT.tile([P, NCB, S], bf16, name="kin", tag="kin")
        kin_r = kT.rearrange("(cb p) n -> p cb n", p=P)
        nc.sync.dma_start(out=wk_sb[:, 0], in_=wk[0])
        nc.sync.dma_start(out=kin[:, 0:4, :], in_=kin_r[:, 0:4, :])
        nc.sync.dma_start(out=kin[:, 4:8, :], in_=kin_r[:, 4:8, :])
        qk_tiles["k"] = [kin[:, cb, :] for cb in range(NCB)]
        # HAM warm-up: a short train of tiny matmuls before qT lands keeps
        # the PE clock-gate at 8/8 so the real stream starts warm
        warm_ps = psum.tile([DH, P], f32, name="warm_ps", tag="mm")
        for _ in range(30):
            nc.tensor.matmul(
                warm_ps, lhsT=ones_k1[0:1, 0:DH], rhs=ones_k1, start=True, stop=True
            )


        def qk_half_burst(pr, tname, ic):
            """8 matmuls accumulating one [128, 512] half of q/k projection."""
            w_sb, b_sb, dst = (
                (wq_sb, bq_sb, qhT) if tname == "q" else (wk_sb, bk_sb, khT)
            )
            ps = psum.tile([P, S], f32, name=f"{tname}p{pr}_{ic}", tag="mm")
            for cb in range(NCB):
                nc.tensor.matmul(
                    ps[:, 0:512],
                    lhsT=w_sb[:, pr, cb, :],
                    rhs=qk_tiles[tname][cb][:, ic * 512:(ic + 1) * 512],
                    start=(cb == 0),
                    stop=(cb == NCB - 1),
                )
            nc.vector.tensor_scalar_add(
                dst[pr][:, ic * 512:(ic + 1) * 512], ps[:, 0:512], b_sb[:, pr:pr + 1]
            )

        def v_burst(jb):
            ps = psum.tile([P, S], f32, name=f"vp{jb}", tag="mm")
            for cb in range(NCB):
                nc.tensor.matmul(
                    ps[:, 0:GCOL],
                    lhsT=vtiles[cb][:, jb * P:(jb + 1) * P],
                    rhs=wv_sb[:, cb, :],
                    start=(cb == 0),
                    stop=False,
                )
            nc.tensor.matmul(ps[:, 0:GCOL], lhsT=ones_k1, rhs=bv_sb, start=False, stop=True)
            nc.vector.tensor_copy(
                out=vh_sb[jb][:, :, 0:DH],
                in_=ps[:, 0:GCOL].rearrange("p (h d) -> p h d", d=DH),
            )
            nc.vector.memset(vh_sb[jb][:, :, DH:DH + 1], 1.0)

        def normalize_start(pr, cr, dma=None):
            """Phase A of softmax-normalize: PSUM->SBUF copies (free the cr
            banks quickly) + broadcast DMAs. Returns phase B (recip + mul),
            which the caller defers until the DMAs have completed on the wall
            clock — otherwise the reciprocal blocks the in-order vector queue
            waiting on the DMA semaphore."""
            dma = dma or nc.sync
            cs, rb = {}, {}
            for hl in range(2):
                cs[hl] = work.tile([DH + 1, S], bf16, name=f"cs{pr}{hl}", tag="cs", bufs=2)
                rb[hl] = work.tile([P, S], f32, name=f"rb{pr}{hl}", tag="rb", bufs=2)
            # during filler-laden pairs (pr<2 finish inside attn1/attn2) the
            # tensor engine paces the slots, so ScalarE has slack for half
            # of the cr-freeing copies; pr2's run inside attn3 where ScalarE
            # is the pacer, so keep those on the vector engine.
            if pr < 2:
                nc.scalar.copy(cs[0], cr[0])
                nc.scalar.copy(rb[0][DH:DH + 1, :], cr[0][DH:DH + 1, :])
            else:
                nc.vector.tensor_copy(out=cs[0], in_=cr[0])
                nc.vector.tensor_copy(out=rb[0][DH:DH + 1, :], in_=cr[0][DH:DH + 1, :])
            nc.vector.tensor_copy(out=cs[1], in_=cr[1])
            nc.vector.tensor_copy(out=rb[1][DH:DH + 1, :], in_=cr[1][DH:DH + 1, :])
            # 2-hop broadcast: 1 -> 8 partitions, then 8 -> 56 (port-parallel;
            # a single 1 -> 64 replication would serialize on one SBUF port)
            for hl in range(2):
                row = rb[hl][DH:DH + 1, :]
                row8 = bass.AP(
                    tensor=row.tensor,
                    offset=row.offset,
                    ap=[list(row.ap[0]), [0, 8]] + [list(d) for d in row.ap[1:]],
                )
                dma.dma_start(out=rb[hl][0:8, :], in_=row8)
            for hl in range(2):
                blk = rb[hl][0:8, :]
                blk_rep = bass.AP(
                    tensor=blk.tensor,
                    offset=blk.offset,
                    ap=[list(blk.ap[0]), [0, 7]] + [list(d) for d in blk.ap[1:]],
                )
                dma.dma_start(out=rb[hl][8:DH, :], in_=blk_rep)

            def finish(hl):
                nc.vector.reciprocal_approx_fast(rb[hl][0:DH, :], rb[hl][0:DH, :])
                if hl == 0:
                    nc.vector.tensor_mul(
                        ctxn[pr][0:DH, :], cs[hl][0:DH, :], rb[hl][0:DH, :]
                    )
                else:
                    ch = work.tile([DH, S], bf16, name=f"ch{pr}", tag="ch", bufs=2)
                    nc.vector.tensor_mul(ch, cs[hl][0:DH, :], rb[hl][0:DH, :])
                    dma.dma_start(out=ctxn[pr][DH:2 * DH, :], in_=ch)

            return [lambda: finish(0), lambda: finish(1)]

        def normalize3(cr):
            """pr3 normalize, tail-optimized: one shared partition-shift DMA,
            reciprocal at partition 0, partition-broadcast via tiny tensor
            matmuls into freed PSUM, and no hl1 shift (outproj reads the two
            64-row halves directly via split wo tiles)."""
            rb = work.tile([P, 2 * S], f32, name="rb3", tag="rb3", bufs=1)
            nc.scalar.copy(rb[DH:DH + 1, 0:S], cr[0][DH:DH + 1, :])
            nc.vector.tensor_copy(out=rb[DH:DH + 1, S:2 * S], in_=cr[1][DH:DH + 1, :])
            cs = {}
            for hl in range(2):
                cs[hl] = work.tile([DH + 1, S], bf16, name=f"cs3{hl}", tag="cs", bufs=2)
            nc.scalar.copy(cs[0], cr[0])
            nc.vector.tensor_copy(out=cs[1], in_=cr[1])
            nc.sync.dma_start(out=rb[0:1, :], in_=rb[DH:DH + 1, :])
            nc.vector.reciprocal_approx_fast(rb[0:1, :], rb[0:1, :])
            ri = work.tile([1, 2 * S], bf16, name="ri3", tag="ri3", bufs=1)
            nc.vector.tensor_copy(out=ri, in_=rb[0:1, :])
            rbc = {}
            for hl in range(2):
                rbc[hl] = psum.tile([DH, S], f32, name=f"rbc3{hl}", tag="cr", bufs=2)
                for ic in range(2):
                    nc.tensor.matmul(
                        rbc[hl][:, ic * 512:(ic + 1) * 512],
                        lhsT=ones_k1[0:1, 0:DH],
                        rhs=ri[0:1, hl * S + ic * 512:hl * S + (ic + 1) * 512],
                        start=True,
                        stop=True,
                    )
            out_lo_hi = []
            for hl in range(2):
                t = work.tile([DH, S], bf16, name=f"cx3{hl}", tag="ch", bufs=2)
                nc.vector.tensor_mul(t, cs[hl][0:DH, :], rbc[hl])
                out_lo_hi.append(t)
            return out_lo_hi

        def attention_pair(pr, filler, ctx_lag=2, preamble=None, pending_pr=None):
            """Attention for head pair pr; `filler` is a list of zero-arg
            callables emitting tensor-engine bursts, interleaved one per key
            block to keep the in-order tensor queue busy while ScalarE exps.
            ctx matmuls lag the scores stream by ctx_lag (jb, hl) units.
            `preamble` (the previous pair's ctx drain + normalize copies) is
            emitted after jb0 so it hides behind this pair's first scores;
            `pending` (the previous pair's deferred normalize finish) at
            jb==2, once its broadcast DMAs have completed on the wall clock.
            Returns (cr accumulator dict, ctx drain closure)."""
            cr = {}
            for hl in range(2):
                cr[hl] = psum.tile(
                    [DH + 1, S], f32, name=f"cr{pr}_{hl}", tag="cr", bufs=2
                )
            ctx_queue = []

            def emit_ctx(jb, hl, e):
                h = 2 * pr + hl
                for ic in range(2):
                    nc.tensor.matmul(
                        cr[hl][:, ic * 512:(ic + 1) * 512],
                        lhsT=vh_sb[jb][:, h, :],
                        rhs=e[:, ic * 512:(ic + 1) * 512],
                        start=(jb == 0),
                        stop=(jb == NJB - 1),
                    )

            for jb in range(NJB):
                # filler burst first: its DVE drain lands ahead of this slot's
                # attention muls in the in-order vector queue
                if filler:
                    filler.pop(0)()
                if jb == 1 and preamble is not None:
                    preamble()
                    preamble = None
                if jb in (3, 5) and pending_pr is not None and fin.get(pending_pr):
                    fin[pending_pr].pop(0)()
                # prefetch exp(bias) for both heads of this key block (one DMA)
                eb = work.tile([P, 2, S], bf16, name=f"eb{pr}_{jb}", tag="eb", bufs=4)
                nc.sync.dma_start(
                    out=eb, in_=expb[pr, jb].rearrange("hl j i -> j hl i")
                )
                ebs = [eb[:, 0, :], eb[:, 1, :]]
                # scores: alternate PE row groups (hl0 rows 0-63 / hl1 rows
                # 64-127) so each LDWEIGHTS hides under the other group's MM
                s_ps = [
                    psum.tile([P, S], f32, name=f"s{2 * pr + hl}_{jb}", tag="mm")
                    for hl in range(2)
                ]
                for ic in range(2):
                    for hl in range(2):
                        nc.tensor.matmul(
                            s_ps[hl][:, ic * 512:(ic + 1) * 512],
                            lhsT=khT[pr][hl * DH:(hl + 1) * DH, jb * P:(jb + 1) * P],
                            rhs=qhT[pr][hl * DH:(hl + 1) * DH, ic * 512:(ic + 1) * 512],
                            start=True,
                            stop=True,
                        )
                for hl in range(2):
                    h = 2 * pr + hl
                    es = work.tile([P, S], bf16, name=f"es{h}_{jb}", tag="es", bufs=3)
                    nc.scalar.activation(es, s_ps[hl], AF.Exp)
                    e = work.tile([P, S], bf16, name=f"e{h}_{jb}", tag="e", bufs=6)
                    nc.vector.tensor_mul(e, es, ebs[hl])
                    ctx_queue.append((jb, hl, e))
                    if len(ctx_queue) > ctx_lag:
                        emit_ctx(*ctx_queue.pop(0))
            while pending_pr is not None and fin.get(pending_pr):
                fin[pending_pr].pop(0)()

            def drain():
                while ctx_queue:
                    emit_ctx(*ctx_queue.pop(0))

            return cr, drain

        # ---- schedule ----
        # pr0 q bursts first (qT lands first), v and k interleaved by their
        # data arrival
        qk_half_burst(0, "q", 0)
        qk_half_burst(0, "q", 1)
        v_burst(0)
        v_burst(1)
        v_burst(2)
        qk_half_burst(0, "k", 0)
        qk_half_burst(0, "k", 1)
        v_burst(3)
        v_burst(4)
        v_burst(5)
        v_burst(6)
        v_burst(7)

        def qk_fillers(pr):
            def load_w(pr=pr):
                nc.sync.dma_start(out=wq_sb[:, pr], in_=wq[pr])
                nc.sync.dma_start(out=wk_sb[:, pr], in_=wk[pr])

            return [load_w] + [
                (lambda pr=pr, t=t, ic=ic: qk_half_burst(pr, t, ic))
                for t in ("q", "k")
                for ic in range(2)
            ]

        cr0, drain0 = attention_pair(0, qk_fillers(1), ctx_lag=3)
        # wo loads sit behind attn0's exp(bias) stream on the sync ring —
        # they are only needed by the output projection at the very end.
        # pr3's wo halves load to base partition 0 so its outproj can read the
        # two normalized 64-row ctx halves without a partition-shift DMA.
        for pr in range(3):
            nc.sync.dma_start(
                out=wo_sb[:, pr, :],
                in_=wo.rearrange("(pr p) n -> p pr n", p=P)[:, pr, :],
            )
        wo3 = [const.tile([DH, HID], bf16, name=f"wo3_{h}", tag=f"wo3_{h}") for h in range(2)]
        nc.sync.dma_start(out=wo3[0], in_=wo[3 * P:3 * P + DH, :])
        nc.sync.dma_start(out=wo3[1], in_=wo[3 * P + DH:4 * P, :])

        fin = {}

        def preamble(pr, cr, dr):
            def go():
                dr()
                fin[pr] = normalize_start(pr, cr)
            return go

        cr1, drain1 = attention_pair(
            1, qk_fillers(2), preamble=preamble(0, cr0, drain0), pending_pr=0
        )
        cr2, drain2 = attention_pair(
            2, qk_fillers(3), preamble=preamble(1, cr1, drain1), pending_pr=1
        )
        cr3, drain3 = attention_pair(
            3, [], ctx_lag=3, preamble=preamble(2, cr2, drain2), pending_pr=2
        )

        # ---- output projection ----
        # pr0-2 contributions first: they run while pr3 normalizes.
        def outproj_012(ib):
            yp = psum.tile([P, HID], f32, name=f"yp{ib}", tag="mm")
            outproj_tiles[ib] = yp
            for pr in range(3):
                for cc in range(2):
                    nc.tensor.matmul(
                        yp[:, cc * 512:(cc + 1) * 512],
                        lhsT=ctxn[pr][:, ib * P:(ib + 1) * P],
                        rhs=wo_sb[:, pr, cc * 512:(cc + 1) * 512],
                        start=(pr == 0),
                        stop=False,
                    )

        def outproj_3(ib, lo_hi):
            yp = outproj_tiles.pop(ib)
            for cc in range(2):
                for h in range(2):
                    nc.tensor.matmul(
                        yp[:, cc * 512:(cc + 1) * 512],
                        lhsT=lo_hi[h][:, ib * P:(ib + 1) * P],
                        rhs=wo3[h][:, cc * 512:(cc + 1) * 512],
                        start=False,
                        stop=(cc == 1 and h == 1),
                    )
            y_sb = outp.tile([P, HID], f32, name=f"y{ib}", tag="y")
            nc.scalar.copy(y_sb, yp)
            nc.sync.dma_start(out=out[ib * P:(ib + 1) * P, :], in_=y_sb)

        outproj_tiles = {}
        # pr0-2 matmuls for the first two blocks depend only on long-ready
        # data: run them during attn3's serial exp->mul->ctx tail
        outproj_012(0)
        outproj_012(1)
        drain3()
        lo_hi = normalize3(cr3)
        outproj_3(0, lo_hi)
        for ib in range(2, NIB):
            outproj_012(ib)
            outproj_3(ib - 1, lo_hi)
        outproj_3(NIB - 1, lo_hi)

    nc.compile()
    return nc


def _get_nc():
    global _CACHED_NC
    if _CACHED_NC is None:
        _CACHED_NC = _build_nc()
    return _CACHED_NC


def _chunk_w(w):
    """[HID, GCOL] -> [pr, p, cb, m] contiguous per-head-pair weight chunks."""
    return np.ascontiguousarray(
        w.reshape(NCB, P, NPAIR, P).transpose(2, 1, 0, 3)
    )


def make_in_maps(q, k, v, attn_bias, Wq, Wk, Wv, Wo, bq, bk, bv, bo):
    scale = DH ** (-0.5)
    in_maps = []
    for core in range(8):
        b, g = divmod(core, 2)
        gs = slice(g * GCOL, (g + 1) * GCOL)
        in_maps.append({
            "qT": np.ascontiguousarray(q[b].T).astype(BF16),
            "kT": np.ascontiguousarray(k[b].T).astype(BF16),
            "vT": np.ascontiguousarray(v[b].T).astype(BF16),
            "wq": _chunk_w((Wq[:, gs] * scale).astype(BF16)),
            "wk": _chunk_w(Wk[:, gs].astype(BF16)),
            "wv": np.ascontiguousarray(Wv[:, gs]).astype(BF16),
            "wo": np.ascontiguousarray(Wo[gs, :]).astype(BF16),
            "bq": (bq[gs] * scale).astype(np.float32),
            "bk": np.ascontiguousarray(bk[gs]).astype(np.float32),
            "bv": np.ascontiguousarray(bv[gs]).astype(BF16),
            "expb": np.ascontiguousarray(
                np.exp(attn_bias[b, g * 8:(g + 1) * 8].transpose(0, 2, 1))
                .astype(BF16)
                .reshape(NPAIR, 2, NJB, P, S)
                .transpose(0, 2, 1, 3, 4)
            ),
        })
    return in_maps


def kernel(q, k, v, attn_bias, Wq, Wk, Wv, Wo, bq, bk, bv, bo, _trace=False):
    from concourse.bass_utils import run_bass_kernel_spmd

    args = [np.asarray(x, dtype=np.float32) for x in
            (q, k, v, attn_bias, Wq, Wk, Wv, Wo, bq, bk, bv, bo)]
    q, k, v, attn_bias, Wq, Wk, Wv, Wo, bq, bk, bv, bo = args
    nc = _get_nc()
    in_maps = make_in_maps(q, k, v, attn_bias, Wq, Wk, Wv, Wo, bq, bk, bv, bo)
    res = run_bass_kernel_spmd(nc, in_maps, core_ids=list(range(8)), trace=_trace)
    y = np.zeros((4, S, HID), np.float32)
    for core in range(8):
        y[core // 2] += res.results[core]["out"]
    y += bo
    if _trace:
        kernel.last_results = res
    return y


# revision 79
# speedup vs baseline: 1.0537x; 1.0537x over previous
"""Multi-head attention (B=4, S=1024, H=1024, heads=16) on 8 trn2 NeuronCores.

Sharding: data-parallel over batch (4) x tensor-parallel over head-groups (2).
Core c handles batch c//2, heads [8*(c%2), 8*(c%2)+8).

Schedule (all engines are in-order, so emission order is the schedule):
  - few, large DMAs (whole-tensor loads; ~0.6us sync-ring slot per DMA)
    ordered by need: vT/wv -> wq[pr0]/qT -> wk[pr0]/kT -> exp(bias) stream;
    later pairs' weight chunks load lazily from inside the attention loop.
  - a train of tiny warm-up matmuls during the initial DMA wait holds the
    PE HAM clock-gate at 8/8 so the real stream starts at 2.4 GHz.
  - per-pair attention slots: scores (row-group alternated so LDWEIGHTS
    hides), one exp + one multiply per [128,1024] tile, ctx matmuls lagged
    behind the exp/mul chain; projection bursts for the NEXT pair fill the
    tensor queue inside the slots.
  - pair boundaries: the previous pair's trailing ctx + normalize copies
    are emitted after the next pair's first scores; reciprocal+muls are
    deferred further so DMA-broadcast latency never blocks the vector queue.
  - output projection accumulates pr0-2 first (hiding pr3's normalize),
    and pr3 contributes via two base-0 K=64 halves (split wo tiles) so no
    partition-shift DMA sits on the critical tail.

Per-core math (all matmuls bf16 with fp32 PSUM accumulation):
  - projections: qh_T/kh_T in [d, i] layout (head dim on partitions), vh in
    [j, hd] layout augmented with a ones column per head (softmax
    denominator comes for free from the ctx matmul).
  - scores computed transposed (keys on partitions): s_T = khT^T @ qhT,
    exp on ScalarE, multiplied by host-precomputed exp(attn_bias)^T.
  - normalize with reciprocal of the denominator row + DMA partition
    broadcast, output projection with row-parallel Wo; host adds the two
    partial results + bo.

Scale (1/8) is folded into Wq/bq on the host. Softmax max-subtraction is
skipped: scores+bias are within +-8 so exp is well-conditioned in fp32.
"""

import numpy as np
import ml_dtypes

BF16 = ml_dtypes.bfloat16

S = 1024
HID = 1024
GCOL = 512  # hidden cols per core (8 heads * 64)
DH = 64
P = 128
NPAIR = 4  # head pairs per core
NJB = 8  # key blocks of 128
NCB = 8  # contraction blocks of 128
NIB = 8  # query blocks of 128

_CACHED_NC = None


def _build_nc():
    import concourse.bass as bass
    import concourse.mybir as mybir
    import concourse.tile as tile
    from concourse import bacc
    from contextlib import ExitStack

    f32 = mybir.dt.float32
    bf16 = mybir.dt.bfloat16
    AF = mybir.ActivationFunctionType

    nc = bacc.Bacc(
        "TRN2",
        target_bir_lowering=False,
        debug=False,
        enable_asserts=False,
        num_devices=8,
    )

    qT = nc.dram_tensor("qT", [HID, S], bf16, kind="ExternalInput").ap()
    kT = nc.dram_tensor("kT", [HID, S], bf16, kind="ExternalInput").ap()
    vT = nc.dram_tensor("vT", [HID, S], bf16, kind="ExternalInput").ap()
    # wq/wk host-chunked as [pr, p, cb, m] so per-head-pair weight loads are
    # contiguous 2KB-per-partition transfers (pr0's slice can load first)
    wq = nc.dram_tensor("wq", [NPAIR, P, NCB, P], bf16, kind="ExternalInput").ap()
    wk = nc.dram_tensor("wk", [NPAIR, P, NCB, P], bf16, kind="ExternalInput").ap()
    wv = nc.dram_tensor("wv", [HID, GCOL], bf16, kind="ExternalInput").ap()
    wo = nc.dram_tensor("wo", [GCOL, HID], bf16, kind="ExternalInput").ap()
    bq = nc.dram_tensor("bq", [GCOL], f32, kind="ExternalInput").ap()
    bk = nc.dram_tensor("bk", [GCOL], f32, kind="ExternalInput").ap()
    bv = nc.dram_tensor("bv", [GCOL], bf16, kind="ExternalInput").ap()
    # exp(bias) host-chunked per (pair, key block): one 512KB DMA each
    expb = nc.dram_tensor(
        "expb", [NPAIR, NJB, 2, P, S], bf16, kind="ExternalInput"
    ).ap()
    out = nc.dram_tensor("out", [S, HID], f32, kind="ExternalOutput").ap()

    with tile.TileContext(nc) as tc, ExitStack() as ctx:
        const = ctx.enter_context(tc.tile_pool(name="const", bufs=1))
        inT = ctx.enter_context(tc.tile_pool(name="inT", bufs=1))
        proj = ctx.enter_context(tc.tile_pool(name="proj", bufs=1))
        work = ctx.enter_context(tc.tile_pool(name="work", bufs=6))
        outp = ctx.enter_context(tc.tile_pool(name="outp", bufs=2))
        psum = ctx.enter_context(tc.tile_pool(name="psum", bufs=2, space="PSUM"))

        # ---- constants / weights ----
        wq_sb = const.tile([P, NPAIR, NCB, P], bf16, tag="wq")
        wk_sb = const.tile([P, NPAIR, NCB, P], bf16, tag="wk")
        wv_sb = const.tile([P, NCB, GCOL], bf16, tag="wv")
        wo_sb = const.tile([P, 3, HID], bf16, tag="wo")
        wv_r = wv.rearrange("(cb p) n -> p cb n", p=P)
        bq_sb = const.tile([P, NPAIR], f32, tag="bq")
        bk_sb = const.tile([P, NPAIR], f32, tag="bk")
        bv_sb = const.tile([1, GCOL], bf16, tag="bv")
        ones_k1 = const.tile([1, P], bf16, tag="ones_k1")
        nc.vector.memset(ones_k1, 1.0)
        # pre-warm the Exp activation table before the attention phase
        warm = const.tile([1, 16], bf16, tag="warm")
        nc.vector.memset(warm, 0.0)
        nc.scalar.activation(warm, warm, AF.Exp)

        qhT = [proj.tile([P, S], bf16, name=f"qhT{i}", tag=f"qhT{i}") for i in range(NPAIR)]
        khT = [proj.tile([P, S], bf16, name=f"khT{i}", tag=f"khT{i}") for i in range(NPAIR)]
        # vh_sb[jb]: [j in block, head, 65] where col 64 is ones (denominator trick)
        vh_sb = [proj.tile([P, 8, DH + 1], bf16, name=f"vh{i}", tag=f"vh{i}") for i in range(NJB)]
        ctxn = [proj.tile([P, S], bf16, name=f"ctxn{i}", tag=f"ctxn{i}") for i in range(NPAIR)]

        # input DMA order: v first (v_proj then overlaps the q/k stream),
        # then pr0's q/k weight chunks + the full q/k inputs. Whole-tensor
        # transfers: the sync DMA ring costs ~0.6us per instruction, so few
        # big DMAs beat many small ones.
        nc.sync.dma_start(out=bv_sb, in_=bv.rearrange("(a n) -> a n", a=1))
        nc.sync.dma_start(out=bq_sb, in_=bq.rearrange("(pr p) -> p pr", p=P))
        nc.sync.dma_start(out=bk_sb, in_=bk.rearrange("(pr p) -> p pr", p=P))
        nc.sync.dma_start(out=wv_sb, in_=wv.rearrange("(cb p) n -> p cb n", p=P))
        vin = inT.tile([P, NCB, S], bf16, name="vin", tag="vin")
        vin_r = vT.rearrange("(cb p) n -> p cb n", p=P)
        nc.sync.dma_start(out=vin[:, 0:4, :], in_=vin_r[:, 0:4, :])
        nc.sync.dma_start(out=vin[:, 4:8, :], in_=vin_r[:, 4:8, :])
        vtiles = [vin[:, cb, :] for cb in range(NCB)]
        qk_tiles = {}
        for tname, src, w_d, w_sb in (("q", qT, wq, wq_sb), ("k", kT, wk, wk_sb)):
            nc.sync.dma_start(out=w_sb[:, 0], in_=w_d[0])
            tin = inT.tile([P, NCB, S], bf16, name=f"{tname}in", tag=f"{tname}in")
            tin_r = src.rearrange("(cb p) n -> p cb n", p=P)
            nc.sync.dma_start(out=tin[:, 0:4, :], in_=tin_r[:, 0:4, :])
            nc.sync.dma_start(out=tin[:, 4:8, :], in_=tin_r[:, 4:8, :])
            qk_tiles[tname] = [tin[:, cb, :] for cb in range(NCB)]
        # HAM warm-up: a train of tiny matmuls during the initial DMA wait
        # keeps the PE clock-gate at 8/8 so real matmuls start warm
        warm_ps = psum.tile([DH, P], f32, name="warm_ps", tag="mm")
        for _ in range(70):
            nc.tensor.matmul(
                warm_ps, lhsT=ones_k1[0:1, 0:DH], rhs=ones_k1, start=True, stop=True
            )


        def qk_half_burst(pr, tname, ic):
            """8 matmuls accumulating one [128, 512] half of q/k projection."""
            w_sb, b_sb, dst = (
                (wq_sb, bq_sb, qhT) if tname == "q" else (wk_sb, bk_sb, khT)
            )
            ps = psum.tile([P, S], f32, name=f"{tname}p{pr}_{ic}", tag="mm")
            for cb in range(NCB):
                nc.tensor.matmul(
                    ps[:, 0:512],
                    lhsT=w_sb[:, pr, cb, :],
                    rhs=qk_tiles[tname][cb][:, ic * 512:(ic + 1) * 512],
                    start=(cb == 0),
                    stop=(cb == NCB - 1),
                )
            nc.vector.tensor_scalar_add(
                dst[pr][:, ic * 512:(ic + 1) * 512], ps[:, 0:512], b_sb[:, pr:pr + 1]
            )

        def v_burst(jb):
            ps = psum.tile([P, S], f32, name=f"vp{jb}", tag="mm")
            for cb in range(NCB):
                nc.tensor.matmul(
                    ps[:, 0:GCOL],
                    lhsT=vtiles[cb][:, jb * P:(jb + 1) * P],
                    rhs=wv_sb[:, cb, :],
                    start=(cb == 0),
                    stop=False,
                )
            nc.tensor.matmul(ps[:, 0:GCOL], lhsT=ones_k1, rhs=bv_sb, start=False, stop=True)
            nc.vector.tensor_copy(
                out=vh_sb[jb][:, :, 0:DH],
                in_=ps[:, 0:GCOL].rearrange("p (h d) -> p h d", d=DH),
            )
            nc.vector.memset(vh_sb[jb][:, :, DH:DH + 1], 1.0)

        def normalize_start(pr, cr, dma=None):
            """Phase A of softmax-normalize: PSUM->SBUF copies (free the cr
            banks quickly) + broadcast DMAs. Returns phase B (recip + mul),
            which the caller defers until the DMAs have completed on the wall
            clock — otherwise the reciprocal blocks the in-order vector queue
            waiting on the DMA semaphore."""
            dma = dma or nc.sync
            cs, rb = {}, {}
            for hl in range(2):
                cs[hl] = work.tile([DH + 1, S], bf16, name=f"cs{pr}{hl}", tag="cs", bufs=2)
                rb[hl] = work.tile([P, S], f32, name=f"rb{pr}{hl}", tag="rb", bufs=2)
            # during filler-laden pairs (pr<2 finish inside attn1/attn2) the
            # tensor engine paces the slots, so ScalarE has slack for half
            # of the cr-freeing copies; pr2's run inside attn3 where ScalarE
            # is the pacer, so keep those on the vector engine.
            if pr < 2:
                nc.scalar.copy(cs[0], cr[0])
                nc.scalar.copy(rb[0][DH:DH + 1, :], cr[0][DH:DH + 1, :])
            else:
                nc.vector.tensor_copy(out=cs[0], in_=cr[0])
                nc.vector.tensor_copy(out=rb[0][DH:DH + 1, :], in_=cr[0][DH:DH + 1, :])
            nc.vector.tensor_copy(out=cs[1], in_=cr[1])
            nc.vector.tensor_copy(out=rb[1][DH:DH + 1, :], in_=cr[1][DH:DH + 1, :])
            # 2-hop broadcast: 1 -> 8 partitions, then 8 -> 56 (port-parallel;
            # a single 1 -> 64 replication would serialize on one SBUF port)
            for hl in range(2):
                row = rb[hl][DH:DH + 1, :]
                row8 = bass.AP(
                    tensor=row.tensor,
                    offset=row.offset,
                    ap=[list(row.ap[0]), [0, 8]] + [list(d) for d in row.ap[1:]],
                )
                dma.dma_start(out=rb[hl][0:8, :], in_=row8)
            for hl in range(2):
                blk = rb[hl][0:8, :]
                blk_rep = bass.AP(
                    tensor=blk.tensor,
                    offset=blk.offset,
                    ap=[list(blk.ap[0]), [0, 7]] + [list(d) for d in blk.ap[1:]],
                )
                dma.dma_start(out=rb[hl][8:DH, :], in_=blk_rep)

            def finish(hl):
                nc.vector.reciprocal_approx_fast(rb[hl][0:DH, :], rb[hl][0:DH, :])
                if hl == 0:
                    nc.vector.tensor_mul(
                        ctxn[pr][0:DH, :], cs[hl][0:DH, :], rb[hl][0:DH, :]
                    )
                else:
                    ch = work.tile([DH, S], bf16, name=f"ch{pr}", tag="ch", bufs=2)
                    nc.vector.tensor_mul(ch, cs[hl][0:DH, :], rb[hl][0:DH, :])
                    dma.dma_start(out=ctxn[pr][DH:2 * DH, :], in_=ch)

            return [lambda: finish(0), lambda: finish(1)]

        def normalize3(cr):
            """pr3 normalize, tail-optimized: one shared partition-shift DMA,
            reciprocal at partition 0, partition-broadcast via tiny tensor
            matmuls into freed PSUM, and no hl1 shift (outproj reads the two
            64-row halves directly via split wo tiles)."""
            rb = work.tile([P, 2 * S], f32, name="rb3", tag="rb3", bufs=1)
            nc.scalar.copy(rb[DH:DH + 1, 0:S], cr[0][DH:DH + 1, :])
            nc.vector.tensor_copy(out=rb[DH:DH + 1, S:2 * S], in_=cr[1][DH:DH + 1, :])
            cs = {}
            for hl in range(2):
                cs[hl] = work.tile([DH + 1, S], bf16, name=f"cs3{hl}", tag="cs", bufs=2)
            nc.scalar.copy(cs[0], cr[0])
            nc.vector.tensor_copy(out=cs[1], in_=cr[1])
            nc.sync.dma_start(out=rb[0:1, :], in_=rb[DH:DH + 1, :])
            nc.vector.reciprocal_approx_fast(rb[0:1, :], rb[0:1, :])
            ri = work.tile([1, 2 * S], bf16, name="ri3", tag="ri3", bufs=1)
            nc.vector.tensor_copy(out=ri, in_=rb[0:1, :])
            rbc = {}
            for hl in range(2):
                rbc[hl] = psum.tile([DH, S], f32, name=f"rbc3{hl}", tag="cr", bufs=2)
                for ic in range(2):
                    nc.tensor.matmul(
                        rbc[hl][:, ic * 512:(ic + 1) * 512],
                        lhsT=ones_k1[0:1, 0:DH],
                        rhs=ri[0:1, hl * S + ic * 512:hl * S + (ic + 1) * 512],
                        start=True,
                        stop=True,
                    )
            out_lo_hi = []
            for hl in range(2):
                t = work.tile([DH, S], bf16, name=f"cx3{hl}", tag="ch", bufs=2)
                nc.vector.tensor_mul(t, cs[hl][0:DH, :], rbc[hl])
                out_lo_hi.append(t)
            return out_lo_hi

        def attention_pair(pr, filler, ctx_lag=2, preamble=None, pending_pr=None):
            """Attention for head pair pr; `filler` is a list of zero-arg
            callables emitting tensor-engine bursts, interleaved one per key
            block to keep the in-order tensor queue busy while ScalarE exps.
            ctx matmuls lag the scores stream by ctx_lag (jb, hl) units.
            `preamble` (the previous pair's ctx drain + normalize copies) is
            emitted after jb0 so it hides behind this pair's first scores;
            `pending` (the previous pair's deferred normalize finish) at
            jb==2, once its broadcast DMAs have completed on the wall clock.
            Returns (cr accumulator dict, ctx drain closure)."""
            cr = {}
            for hl in range(2):
                cr[hl] = psum.tile(
                    [DH + 1, S], f32, name=f"cr{pr}_{hl}", tag="cr", bufs=2
                )
            ctx_queue = []

            def emit_ctx(jb, hl, e):
                h = 2 * pr + hl
                for ic in range(2):
                    nc.tensor.matmul(
                        cr[hl][:, ic * 512:(ic + 1) * 512],
                        lhsT=vh_sb[jb][:, h, :],
                        rhs=e[:, ic * 512:(ic + 1) * 512],
                        start=(jb == 0),
                        stop=(jb == NJB - 1),
                    )

            for jb in range(NJB):
                # filler burst first: its DVE drain lands ahead of this slot's
                # attention muls in the in-order vector queue
                if filler:
                    filler.pop(0)()
                if jb == 1 and preamble is not None:
                    preamble()
                    preamble = None
                if jb in (3, 5) and pending_pr is not None and fin.get(pending_pr):
                    fin[pending_pr].pop(0)()
                # prefetch exp(bias) for both heads of this key block (one DMA)
                eb = work.tile([P, 2, S], bf16, name=f"eb{pr}_{jb}", tag="eb", bufs=4)
                nc.sync.dma_start(
                    out=eb, in_=expb[pr, jb].rearrange("hl j i -> j hl i")
                )
                ebs = [eb[:, 0, :], eb[:, 1, :]]
                # scores: alternate PE row groups (hl0 rows 0-63 / hl1 rows
                # 64-127) so each LDWEIGHTS hides under the other group's MM
                s_ps = [
                    psum.tile([P, S], f32, name=f"s{2 * pr + hl}_{jb}", tag="mm")
                    for hl in range(2)
                ]
                for ic in range(2):
                    for hl in range(2):
                        nc.tensor.matmul(
                            s_ps[hl][:, ic * 512:(ic + 1) * 512],
                            lhsT=khT[pr][hl * DH:(hl + 1) * DH, jb * P:(jb + 1) * P],
                            rhs=qhT[pr][hl * DH:(hl + 1) * DH, ic * 512:(ic + 1) * 512],
                            start=True,
                            stop=True,
                        )
                for hl in range(2):
                    h = 2 * pr + hl
                    es = work.tile([P, S], bf16, name=f"es{h}_{jb}", tag="es", bufs=3)
                    nc.scalar.activation(es, s_ps[hl], AF.Exp)
                    e = work.tile([P, S], bf16, name=f"e{h}_{jb}", tag="e", bufs=6)
                    nc.vector.tensor_mul(e, es, ebs[hl])
                    ctx_queue.append((jb, hl, e))
                    if len(ctx_queue) > ctx_lag:
                        emit_ctx(*ctx_queue.pop(0))
            while pending_pr is not None and fin.get(pending_pr):
                fin[pending_pr].pop(0)()

            def drain():
                while ctx_queue:
                    emit_ctx(*ctx_queue.pop(0))

            return cr, drain

        # ---- schedule ----
        # v bursts interleaved with pr0 q/k bursts by data-arrival order
        # (vT lands first, then qT, then kT)
        v_burst(0)
        v_burst(1)
        v_burst(2)
        v_burst(3)
        qk_half_burst(0, "q", 0)
        v_burst(4)
        qk_half_burst(0, "q", 1)
        v_burst(5)
        qk_half_burst(0, "k", 0)
        v_burst(6)
        qk_half_burst(0, "k", 1)
        v_burst(7)

        def qk_fillers(pr):
            def load_w(pr=pr):
                nc.sync.dma_start(out=wq_sb[:, pr], in_=wq[pr])
                nc.sync.dma_start(out=wk_sb[:, pr], in_=wk[pr])

            return [load_w] + [
                (lambda pr=pr, t=t, ic=ic: qk_half_burst(pr, t, ic))
                for t in ("q", "k")
                for ic in range(2)
            ]

        cr0, drain0 = attention_pair(0, qk_fillers(1), ctx_lag=3)
        # wo loads sit behind attn0's exp(bias) stream on the sync ring —
        # they are only needed by the output projection at the very end.
        # pr3's wo halves load to base partition 0 so its outproj can read the
        # two normalized 64-row ctx halves without a partition-shift DMA.
        for pr in range(3):
            nc.sync.dma_start(
                out=wo_sb[:, pr, :],
                in_=wo.rearrange("(pr p) n -> p pr n", p=P)[:, pr, :],
            )
        wo3 = [const.tile([DH, HID], bf16, name=f"wo3_{h}", tag=f"wo3_{h}") for h in range(2)]
        nc.sync.dma_start(out=wo3[0], in_=wo[3 * P:3 * P + DH, :])
        nc.sync.dma_start(out=wo3[1], in_=wo[3 * P + DH:4 * P, :])

        fin = {}

        def preamble(pr, cr, dr):
            def go():
                dr()
                fin[pr] = normalize_start(pr, cr)
            return go

        cr1, drain1 = attention_pair(
            1, qk_fillers(2), preamble=preamble(0, cr0, drain0), pending_pr=0
        )
        cr2, drain2 = attention_pair(
            2, qk_fillers(3), preamble=preamble(1, cr1, drain1), pending_pr=1
        )
        cr3, drain3 = attention_pair(
            3, [], ctx_lag=3, preamble=preamble(2, cr2, drain2), pending_pr=2
        )

        # ---- output projection ----
        # pr0-2 contributions first: they run while pr3 normalizes.
        def outproj_012(ib):
            yp = psum.tile([P, HID], f32, name=f"yp{ib}", tag="mm")
            outproj_tiles[ib] = yp
            for pr in range(3):
                for cc in range(2):
                    nc.tensor.matmul(
                        yp[:, cc * 512:(cc + 1) * 512],
                        lhsT=ctxn[pr][:, ib * P:(ib + 1) * P],
                        rhs=wo_sb[:, pr, cc * 512:(cc + 1) * 512],
                        start=(pr == 0),
                        stop=False,
                    )

        def outproj_3(ib, lo_hi):
            yp = outproj_tiles.pop(ib)
            for cc in range(2):
                for h in range(2):
                    nc.tensor.matmul(
                        yp[:, cc * 512:(cc + 1) * 512],
                        lhsT=lo_hi[h][:, ib * P:(ib + 1) * P],
                        rhs=wo3[h][:, cc * 512:(cc + 1) * 512],
                        start=False,
                        stop=(cc == 1 and h == 1),
                    )
            y_sb = outp.tile([P, HID], f32, name=f"y{ib}", tag="y")
            nc.scalar.copy(y_sb, yp)
            nc.sync.dma_start(out=out[ib * P:(ib + 1) * P, :], in_=y_sb)

        outproj_tiles = {}
        # pr0-2 matmuls for the first two blocks depend only on long-ready
        # data: run them during attn3's serial exp->mul->ctx tail
        outproj_012(0)
        outproj_012(1)
        drain3()
        lo_hi = normalize3(cr3)
        outproj_3(0, lo_hi)
        for ib in range(2, NIB):
            outproj_012(ib)
            outproj_3(ib - 1, lo_hi)
        outproj_3(NIB - 1, lo_hi)

    nc.compile()
    return nc


def _get_nc():
    global _CACHED_NC
    if _CACHED_NC is None:
        _CACHED_NC = _build_nc()
    return _CACHED_NC


def _chunk_w(w):
    """[HID, GCOL] -> [pr, p, cb, m] contiguous per-head-pair weight chunks."""
    return np.ascontiguousarray(
        w.reshape(NCB, P, NPAIR, P).transpose(2, 1, 0, 3)
    )


def make_in_maps(q, k, v, attn_bias, Wq, Wk, Wv, Wo, bq, bk, bv, bo):
    scale = DH ** (-0.5)
    in_maps = []
    for core in range(8):
        b, g = divmod(core, 2)
        gs = slice(g * GCOL, (g + 1) * GCOL)
        in_maps.append({
            "qT": np.ascontiguousarray(q[b].T).astype(BF16),
            "kT": np.ascontiguousarray(k[b].T).astype(BF16),
            "vT": np.ascontiguousarray(v[b].T).astype(BF16),
            "wq": _chunk_w((Wq[:, gs] * scale).astype(BF16)),
            "wk": _chunk_w(Wk[:, gs].astype(BF16)),
            "wv": np.ascontiguousarray(Wv[:, gs]).astype(BF16),
            "wo": np.ascontiguousarray(Wo[gs, :]).astype(BF16),
            "bq": (bq[gs] * scale).astype(np.float32),
            "bk": np.ascontiguousarray(bk[gs]).astype(np.float32),
            "bv": np.ascontiguousarray(bv[gs]).astype(BF16),
            "expb": np.ascontiguousarray(
                np.exp(attn_bias[b, g * 8:(g + 1) * 8].transpose(0, 2, 1))
                .astype(BF16)
                .reshape(NPAIR, 2, NJB, P, S)
                .transpose(0, 2, 1, 3, 4)
            ),
        })
    return in_maps


def kernel(q, k, v, attn_bias, Wq, Wk, Wv, Wo, bq, bk, bv, bo, _trace=False):
    from concourse.bass_utils import run_bass_kernel_spmd

    args = [np.asarray(x, dtype=np.float32) for x in
            (q, k, v, attn_bias, Wq, Wk, Wv, Wo, bq, bk, bv, bo)]
    q, k, v, attn_bias, Wq, Wk, Wv, Wo, bq, bk, bv, bo = args
    nc = _get_nc()
    in_maps = make_in_maps(q, k, v, attn_bias, Wq, Wk, Wv, Wo, bq, bk, bv, bo)
    res = run_bass_kernel_spmd(nc, in_maps, core_ids=list(range(8)), trace=_trace)
    y = np.zeros((4, S, HID), np.float32)
    for core in range(8):
        y[core // 2] += res.results[core]["out"]
    y += bo
    if _trace:
        kernel.last_results = res
    return y
